# revision 2
# baseline (speedup 1.0000x reference)
"""Trainium2 Bass kernel for y = -x + (A @ x^2) / (x^2 + 1).

A [16384, 16384] is sharded row-wise across 8 NeuronCores (2048 output
rows per core) and quantized on the host to fp8 e4m3: the dot products
sum 16384 independent rounding errors, so the quantization noise
cancels to ~1e-3 relative — far inside the 2e-2 gate — while cutting
HBM traffic 4x (32 MiB/core). The host also pre-interleaves each
core's transposed slice into the DoubleRow layout
    at8[k, q*(2*ROWS) + t*ROWS + i] = A[c*ROWS + i, (2q+t)*128 + k]
so every DMA is a single fully-contiguous multi-MiB transfer (the
per-transfer ~2us completion stall is what kept the f32 baseline at
76% of the HBM roofline). The matmul runs in fp8 DoubleRow perf mode
(2 K-tiles / 256-deep contraction per pass, 0.5 cycles per moving
row), leaving the PE at ~2x slack under the DMA. The elementwise
epilogue uses an exact f32 copy of x, so only the A@x^2 aggregation
sees fp8 noise.
"""

import numpy as np
import ml_dtypes

import concourse.bacc as bacc
import concourse.tile as tile
from concourse import mybir
from concourse.bass_utils import run_bass_kernel_spmd

N_NODES = 16384
DIM = 16
N_CORES = 8
ROWS = N_NODES // N_CORES      # 2048 output rows per core
P = 128                        # SBUF partitions / matmul contraction tile
JB = N_NODES // P              # 128 contraction blocks
QB = JB // 2                   # 64 DoubleRow pair-blocks
NCHUNK = 512                   # matmul moving free dim (one PSUM bank)
ICN = ROWS // NCHUNK           # 4 output column chunks per core

f32 = mybir.dt.float32
f8 = mybir.dt.float8e4
FP8 = ml_dtypes.float8_e4m3


def build_program(reps: int = 1, a_bufs: int = 3, g_pairs: int = 8):
    """g_pairs = DoubleRow pair-blocks per DMA tile (8 -> 4 MiB DMAs)."""
    assert QB % g_pairs == 0
    n_tiles = QB // g_pairs
    nc = bacc.Bacc(
        "TRN2", target_bir_lowering=False, debug=False, num_devices=N_CORES
    )
    at_d = nc.dram_tensor("at", [P, QB * 2 * ROWS], f8, kind="ExternalInput")
    xq_d = nc.dram_tensor("xq", [P, JB * DIM], f8, kind="ExternalInput")
    xt_d = nc.dram_tensor("xt", [DIM, ROWS], f32, kind="ExternalInput")
    yt_d = nc.dram_tensor("yt", [DIM, ROWS], f32, kind="ExternalOutput")

    with tile.TileContext(nc) as tc:
        with (
            tc.tile_pool(name="const", bufs=1) as const_pool,
            tc.tile_pool(name="a", bufs=a_bufs) as a_pool,
            tc.tile_pool(name="ps", bufs=1, space="PSUM") as ps_pool,
            tc.tile_pool(name="y", bufs=2) as y_pool,
        ):
            # Resident x^2 table (fp8, host-quantized):
            # xh[k, q*32 + t*16 + d] = x[(2q+t)*128 + k, d]^2, so
            # xh[:, q*32:(q+1)*32] viewed [128, 2, 16] is the DoubleRow
            # lhsT for pair-block q.
            xh = const_pool.tile([P, JB * DIM], f8, tag="xh")
            nc.sync.dma_start(xh[:], xq_d.ap())

            # Epilogue constants on the local row slice (transposed, f32):
            # xt[d, i] = x[c*2048 + i, d], rcp = 1 / (xt^2 + 1).
            xt = const_pool.tile([DIM, ROWS], f32, tag="xt")
            nc.sync.dma_start(xt[:], xt_d.ap())
            rcp = const_pool.tile([DIM, ROWS], f32, tag="rcp")
            nc.vector.tensor_mul(rcp[:], xt[:], xt[:])
            nc.scalar.add(rcp[:], rcp[:], 1.0)
            nc.vector.reciprocal(rcp[:], rcp[:])

            tile_free = g_pairs * 2 * ROWS   # fp8 bytes per partition

            def body():
                ps = [
                    ps_pool.tile([DIM, NCHUNK], f32,
                                 name=f"ps{ic}", tag=f"ps{ic}")
                    for ic in range(ICN)
                ]
                for ti in range(n_tiles):
                    a_t = a_pool.tile([P, tile_free], f8, name="a_t", tag="a")
                    eng = nc.sync if ti % 2 == 0 else nc.scalar
                    eng.dma_start(
                        a_t[:],
                        at_d.ap()[:, ti * tile_free:(ti + 1) * tile_free],
                    )
                    a_v = a_t[:].rearrange(
                        "p (g t i) -> p g t i", g=g_pairs, t=2
                    )
                    for g in range(g_pairs):
                        q = ti * g_pairs + g
                        lhsT = xh[:, q * 2 * DIM:(q + 1) * 2 * DIM].rearrange(
                            "p (t d) -> p t d", t=2
                        )
                        for ic in range(ICN):
                            nc.tensor.matmul(
                                ps[ic][:],
                                lhsT,
                                a_v[:, g, :,
                                    ic * NCHUNK:(ic + 1) * NCHUNK],
                                start=(q == 0),
                                stop=(q == QB - 1),
                                perf_mode=mybir.MatmulPerfMode.DoubleRow,
                            )
                for ic in range(ICN):
                    sl = slice(ic * NCHUNK, (ic + 1) * NCHUNK)
                    y_t = y_pool.tile([DIM, NCHUNK], f32, name="y_t", tag="y")
                    nc.vector.tensor_mul(y_t[:], ps[ic][:], rcp[:, sl])
                    nc.vector.tensor_sub(y_t[:], y_t[:], xt[:, sl])
                    nc.sync.dma_start(yt_d.ap()[:, sl], y_t[:])

            if reps == 1:
                body()
            else:
                with tc.For_i(0, reps, 1):
                    body()
    nc.compile()
    return nc


def shard_inputs(A: np.ndarray, x: np.ndarray) -> list[dict]:
    A = np.ascontiguousarray(A, dtype=np.float32)
    x = np.ascontiguousarray(x, dtype=np.float32)
    A8 = A.astype(FP8)
    xh8 = (x * x).astype(FP8)
    # xq8[k, q*32 + t*16 + d] = xh[(2q+t)*128 + k, d]
    xq8 = np.ascontiguousarray(
        xh8.reshape(QB, 2, P, DIM).transpose(2, 0, 1, 3)
    ).reshape(P, JB * DIM)
    in_maps = []
    for c in range(N_CORES):
        rows = slice(c * ROWS, (c + 1) * ROWS)
        # at8[k, q*(2*ROWS) + t*ROWS + i] = A[c*ROWS + i, (2q+t)*128 + k]
        at8 = np.ascontiguousarray(
            A8[rows, :].reshape(ROWS, QB, 2, P).transpose(3, 1, 2, 0)
        ).reshape(P, QB * 2 * ROWS)
        in_maps.append({
            "at": at8,
            "xq": xq8,
            "xt": np.ascontiguousarray(x[rows, :].T),
        })
    return in_maps


def gather_output(results: list[dict]) -> np.ndarray:
    return np.concatenate(
        [np.asarray(results[c]["yt"]).T for c in range(N_CORES)], axis=0
    ).astype(np.float32)


def kernel(A, x, t=None, **_unused) -> np.ndarray:
    nc = build_program(reps=1)
    in_maps = shard_inputs(np.asarray(A), np.asarray(x))
    res = run_bass_kernel_spmd(nc, in_maps, core_ids=list(range(N_CORES)))
    return gather_output(res.results)


# revision 6
# speedup vs baseline: 6.2946x; 6.2946x over previous
"""Trainium2 Bass kernel for y = -x + (A @ x^2) / (x^2 + 1).

A [16384, 16384] is sharded row-wise across 8 NeuronCores (2048 output
rows per core) and quantized on the host to fp8 e4m3: the dot products
sum 16384 independent rounding errors, so the quantization noise
cancels to ~1e-3 relative — far inside the 2e-2 gate — while cutting
HBM traffic 4x (32 MiB/core). The host also pre-interleaves each
core's transposed slice into the DoubleRow layout
    at8[k, q*(2*ROWS) + t*ROWS + i] = A[c*ROWS + i, (2q+t)*128 + k]
so every DMA is a single fully-contiguous multi-MiB transfer (the
per-transfer ~2us completion stall is what kept the f32 baseline at
76% of the HBM roofline). The matmul runs in fp8 DoubleRow perf mode
(2 K-tiles / 256-deep contraction per pass, 0.5 cycles per moving
row), leaving the PE at ~2x slack under the DMA. The elementwise
epilogue uses an exact f32 copy of x, so only the A@x^2 aggregation
sees fp8 noise.
"""

import numpy as np
import ml_dtypes

import concourse.bacc as bacc
import concourse.tile as tile
from concourse import mybir
from concourse.bass_utils import run_bass_kernel_spmd

N_NODES = 16384
DIM = 16
N_CORES = 8
ROWS = N_NODES // N_CORES      # 2048 output rows per core
P = 128                        # SBUF partitions / matmul contraction tile
JB = N_NODES // P              # 128 contraction blocks
QB = JB // 2                   # 64 DoubleRow pair-blocks
NCHUNK = 512                   # matmul moving free dim (one PSUM bank)
ICN = ROWS // NCHUNK           # 4 output column chunks per core

f32 = mybir.dt.float32
f8 = mybir.dt.float8e4
FP8 = ml_dtypes.float8_e4m3


def build_program(reps: int = 1, a_bufs: int = 6, g_pairs: int = 4,
                  alt_queues: bool = False, unroll: bool = False):
    """g_pairs = DoubleRow pair-blocks per DMA tile (4 -> 2 MiB DMAs)."""
    assert QB % g_pairs == 0
    n_tiles = QB // g_pairs
    nc = bacc.Bacc(
        "TRN2", target_bir_lowering=False, debug=False, num_devices=N_CORES
    )
    at_d = nc.dram_tensor("at", [P, QB * 2 * ROWS], f8, kind="ExternalInput")
    xq_d = nc.dram_tensor("xq", [P, JB * DIM], f8, kind="ExternalInput")
    xt_d = nc.dram_tensor("xt", [DIM, ROWS], f32, kind="ExternalInput")
    yt_d = nc.dram_tensor("yt", [DIM, ROWS], f32, kind="ExternalOutput")

    with tile.TileContext(nc) as tc:
        with (
            tc.tile_pool(name="const", bufs=1) as const_pool,
            tc.tile_pool(name="a", bufs=a_bufs) as a_pool,
            tc.tile_pool(name="ps", bufs=1, space="PSUM") as ps_pool,
            tc.tile_pool(name="y", bufs=2) as y_pool,
        ):
            # Resident x^2 table (fp8, host-quantized):
            # xh[k, q*32 + t*16 + d] = x[(2q+t)*128 + k, d]^2, so
            # xh[:, q*32:(q+1)*32] viewed [128, 2, 16] is the DoubleRow
            # lhsT for pair-block q.
            xh = const_pool.tile([P, JB * DIM], f8, tag="xh")
            nc.sync.dma_start(xh[:], xq_d.ap())

            # Epilogue constants on the local row slice (transposed, f32):
            # xt[d, i] = x[c*2048 + i, d], rcp = 1 / (xt^2 + 1).
            xt = const_pool.tile([DIM, ROWS], f32, tag="xt")
            nc.sync.dma_start(xt[:], xt_d.ap())
            rcp = const_pool.tile([DIM, ROWS], f32, tag="rcp")
            nc.vector.tensor_mul(rcp[:], xt[:], xt[:])
            nc.scalar.add(rcp[:], rcp[:], 1.0)
            nc.vector.reciprocal(rcp[:], rcp[:])

            tile_free = g_pairs * 2 * ROWS   # fp8 bytes per partition

            def body():
                ps = [
                    ps_pool.tile([DIM, NCHUNK], f32,
                                 name=f"ps{ic}", tag=f"ps{ic}")
                    for ic in range(ICN)
                ]
                for ti in range(n_tiles):
                    a_t = a_pool.tile([P, tile_free], f8, name="a_t", tag="a")
                    eng = nc.sync if (ti % 2 == 0 or not alt_queues) \
                        else nc.scalar
                    eng.dma_start(
                        a_t[:],
                        at_d.ap()[:, ti * tile_free:(ti + 1) * tile_free],
                    )
                    a_v = a_t[:].rearrange(
                        "p (g t i) -> p g t i", g=g_pairs, t=2
                    )
                    for g in range(g_pairs):
                        q = ti * g_pairs + g
                        lhsT = xh[:, q * 2 * DIM:(q + 1) * 2 * DIM].rearrange(
                            "p (t d) -> p t d", t=2
                        )
                        for ic in range(ICN):
                            nc.tensor.matmul(
                                ps[ic][:],
                                lhsT,
                                a_v[:, g, :,
                                    ic * NCHUNK:(ic + 1) * NCHUNK],
                                start=(q == 0),
                                stop=(q == QB - 1),
                                perf_mode=mybir.MatmulPerfMode.DoubleRow,
                            )
                for ic in range(ICN):
                    sl = slice(ic * NCHUNK, (ic + 1) * NCHUNK)
                    y_t = y_pool.tile([DIM, NCHUNK], f32, name="y_t", tag="y")
                    nc.vector.tensor_mul(y_t[:], ps[ic][:], rcp[:, sl])
                    nc.vector.tensor_sub(y_t[:], y_t[:], xt[:, sl])
                    nc.sync.dma_start(yt_d.ap()[:, sl], y_t[:])

            if reps == 1:
                body()
            elif unroll:
                for _ in range(reps):
                    body()
            else:
                with tc.For_i(0, reps, 1):
                    body()
    nc.compile()
    return nc


def shard_inputs(A: np.ndarray, x: np.ndarray) -> list[dict]:
    A = np.ascontiguousarray(A, dtype=np.float32)
    x = np.ascontiguousarray(x, dtype=np.float32)
    A8 = A.astype(FP8)
    xh8 = (x * x).astype(FP8)
    # xq8[k, q*32 + t*16 + d] = xh[(2q+t)*128 + k, d]
    xq8 = np.ascontiguousarray(
        xh8.reshape(QB, 2, P, DIM).transpose(2, 0, 1, 3)
    ).reshape(P, JB * DIM)
    in_maps = []
    for c in range(N_CORES):
        rows = slice(c * ROWS, (c + 1) * ROWS)
        # at8[k, q*(2*ROWS) + t*ROWS + i] = A[c*ROWS + i, (2q+t)*128 + k]
        at8 = np.ascontiguousarray(
            A8[rows, :].reshape(ROWS, QB, 2, P).transpose(3, 1, 2, 0)
        ).reshape(P, QB * 2 * ROWS)
        in_maps.append({
            "at": at8,
            "xq": xq8,
            "xt": np.ascontiguousarray(x[rows, :].T),
        })
    return in_maps


def gather_output(results: list[dict]) -> np.ndarray:
    return np.concatenate(
        [np.asarray(results[c]["yt"]).T for c in range(N_CORES)], axis=0
    ).astype(np.float32)


def kernel(A, x, t=None, **_unused) -> np.ndarray:
    nc = build_program(reps=1)
    in_maps = shard_inputs(np.asarray(A), np.asarray(x))
    res = run_bass_kernel_spmd(nc, in_maps, core_ids=list(range(N_CORES)))
    return gather_output(res.results)
